# revision 38
# baseline (speedup 1.0000x reference)
"""Trainium2 Bass kernel for ViT attention with LSA (per-head scale, masked diag).

Full inputs in, full outputs out. Sharding: data-parallel over batch across
8 NeuronCores (4 batches each). No collectives.

Per-core pipeline (per batch; x host-transposed + token-padded 577 -> 640):
  xT [768, 640]   plain contiguous DMA loads (transpose done on host)
  qT,kT = W_{q,k}^T @ x^T       (bf16 matmul -> bf16 tiles, feature-major,
                                 LSA scale folded into Wq on host)
  v     = x @ W_v               (token-major; vext[jt][h] = [v_h(64) | ones(64)])
  dotsT[j,i] = kT^T qT          (bf16 matmul K=64; heads 2m/2m+1 row-tiled at
                                 PE rows 0-63 / 64-127, issued adjacently so
                                 the two matmuls run concurrently ~2x)
  A = exp(dotsT)                (ACT, PSUM->SBUF bf16; no max-sub needed)
  A[diag block] *= (1 - I_128)  (DVE, only the diagonal 128-window per j-tile)
  oe[128, i] = vext^T A         (bf16 matmul; rows 64-127 = softmax denominator
                                 replicated via the 64 ones columns -- no
                                 cross-partition reduce or broadcast needed)
  attn_out = oe[0:64] * recip(oe[64:128])    (DVE f32 recip + mult -> bf16)
  out = attn_out^T @ W_out + b  (bf16 matmul, token-major; bf16 DMA out,
                                 upcast to f32 on host)

Batches are software-pipelined: batch b+1's projections (and, for the last
batch, all deferred out-projections) interleave into batch b's attention
stream at 9 fill points per head-pair to keep the PE dense (HAM warm).
The odd head's AV accumulator alternates into the pd pool so it never waits
on the even head's normalize chain.
"""

import numpy as np

B, N, DIM = 32, 577, 768
H, DH = 12, 64
NCORES = 8
BPC = B // NCORES          # batches per core
T = N                      # real tokens per batch
NP = 580                   # padded i-width (2 chunks of 290)
CH = 290                   # i-chunk size
XW = 640                   # DMA-transpose width (needs %128 free dim)
TT = [(i * 128, min(128, T - i * 128)) for i in range((T + 127) // 128)]  # j/t tiles

_cache = {}


def _build_nc():
    import concourse.bass as bass
    import concourse.tile as tile
    from concourse import bacc, mybir

    f32 = mybir.dt.float32
    bf16 = mybir.dt.bfloat16
    AF = mybir.ActivationFunctionType
    OP = mybir.AluOpType

    nc = bacc.Bacc("TRN2", target_bir_lowering=False, debug=False)

    # x arrives host-transposed: [batch, feature, token(padded to XW)]
    x_d = nc.dram_tensor("x", [BPC, DIM, XW], bf16, kind="ExternalInput").ap()
    wqkv_d = nc.dram_tensor("wqkv", [DIM, 3 * DIM], bf16, kind="ExternalInput").ap()
    wout_d = nc.dram_tensor("wout", [DIM, DIM], bf16, kind="ExternalInput").ap()
    bout_d = nc.dram_tensor("bout", [DIM], f32, kind="ExternalInput").ap()
    mask_d = nc.dram_tensor("mask", [128, 128], bf16, kind="ExternalInput").ap()
    out_d = nc.dram_tensor("out", [BPC, N, DIM], bf16, kind="ExternalOutput").ap()

    with tile.TileContext(nc) as tc:
        with (
            tc.tile_pool(name="weights", bufs=1) as wp,
            tc.tile_pool(name="xt", bufs=3) as xtp,
            tc.tile_pool(name="qkt", bufs=2) as qkp,
            tc.tile_pool(name="vext", bufs=2) as vp,
            tc.tile_pool(name="aout", bufs=4) as aop,
            tc.tile_pool(name="apool", bufs=6) as apl,
            tc.tile_pool(name="small", bufs=2) as smp,
            tc.tile_pool(name="ostage", bufs=2) as osp,
            tc.tile_pool(name="pmisc", bufs=2, space="PSUM") as pmp,
            tc.tile_pool(name="pdots", bufs=2, space="PSUM") as pdp,
            tc.tile_pool(name="poext", bufs=1, space="PSUM") as pop,
        ):
            # ---- static tiles ----
            wqkv = wp.tile([128, 6, 3 * DIM], bf16)

            # weight loads ride the otherwise-idle gpsimd SWDGE queue so they
            # never serialize behind xT / output traffic or a busy engine FIFO
            def load_wqkv_block(kc, blk, eng=None):
                (eng or nc.gpsimd).dma_start(
                    out=wqkv[:, kc, blk * DIM : (blk + 1) * DIM],
                    in_=wqkv_d[kc * 128 : (kc + 1) * 128, blk * DIM : (blk + 1) * DIM],
                )

            wout = wp.tile([128, 6, DIM], bf16)

            def load_wout():
                for kc in range(6):
                    nc.gpsimd.dma_start(
                        out=wout[:, kc, :], in_=wout_d[kc * 128 : (kc + 1) * 128, :]
                    )

            mask = wp.tile([128, 128], bf16)
            b_bc = wp.tile([128, DIM], f32)

            def load_misc():
                nc.gpsimd.dma_start(out=mask, in_=mask_d)
                bout_bcast = bass.AP(
                    tensor=bout_d.tensor,
                    offset=bout_d.offset,
                    ap=[[0, 128], [1, DIM]],
                )
                nc.gpsimd.dma_start(out=b_bc, in_=bout_bcast)

            state = {}  # b -> dict(xT=, qkT=, vext=)

            def prep_units(b):
                """Thunks for batch b's load/projection work."""
                if b >= BPC:
                    return []
                st = {}
                state[b] = st
                units = []

                def xt_unit():
                    def run():
                        st["xT"] = xtp.tile([128, 6, XW], bf16, tag="xT", name="xT")
                        for dc in range(6):
                            nc.sync.dma_start(
                                out=st["xT"][:, dc, :],
                                in_=x_d[b, dc * 128 : (dc + 1) * 128, :],
                            )

                    return run

                def qk_unit(ft):
                    # c-inner: consecutive matmuls alternate PSUM banks so
                    # drain of one overlaps fill of the other
                    def run():
                        if "qkT" not in st:
                            st["qkT"] = qkp.tile(
                                [128, 12, NP], bf16, tag="qkT", name="qkT"
                            )
                        xT, qkT = st["xT"], st["qkT"]
                        pq = [
                            pmp.tile([128, 512], f32, tag="pm", name="pq0"),
                            pmp.tile([128, 512], f32, tag="pm", name="pq1"),
                        ]
                        for kc in range(6):
                            for c in range(2):
                                nc.tensor.matmul(
                                    pq[c][:, 0:CH],
                                    wqkv[:, kc, ft * 128 : (ft + 1) * 128],
                                    xT[:, kc, c * CH : (c + 1) * CH],
                                    start=(kc == 0),
                                    stop=(kc == 5),
                                )
                        nc.vector.tensor_copy(
                            out=qkT[:, ft, 0:CH], in_=pq[0][:, 0:CH]
                        )
                        nc.scalar.copy(
                            out=qkT[:, ft, CH : 2 * CH], in_=pq[1][:, 0:CH]
                        )

                    return run

                def v_unit(tt, t0, tn):
                    def run():
                        if "vext" not in st:
                            st["vext"] = vp.tile(
                                [128, len(TT), 12, 128], bf16, tag="vext",
                                name="vext",
                            )
                            # ones block: cols 64-127 of every (tt, h) slice;
                            # AV's ones columns replicate the softmax
                            # denominator onto PSUM partitions 64-127.
                            nc.vector.memset(st["vext"][:, :, :, 64:128], 1.0)
                        xT, vext = st["xT"], st["vext"]
                        pv0 = pmp.tile([128, 512], f32, tag="pm", name="pv0")
                        pv1 = pmp.tile([128, 512], f32, tag="pm", name="pv1")
                        for kc in range(6):
                            nc.tensor.matmul(
                                pv0[0:tn, 0:512],
                                xT[:, kc, t0 : t0 + tn],
                                wqkv[:, kc, 1536:2048],
                                start=(kc == 0),
                                stop=(kc == 5),
                            )
                            nc.tensor.matmul(
                                pv1[0:tn, 0:256],
                                xT[:, kc, t0 : t0 + tn],
                                wqkv[:, kc, 2048:2304],
                                start=(kc == 0),
                                stop=(kc == 5),
                            )
                        nc.vector.tensor_copy(
                            out=vext[0:tn, tt, 0:8, 0:DH],
                            in_=pv0[0:tn, 0:512].rearrange("p (h d) -> p h d", h=8),
                        )
                        nc.vector.tensor_copy(
                            out=vext[0:tn, tt, 8:12, 0:DH],
                            in_=pv1[0:tn, 0:256].rearrange("p (h d) -> p h d", h=4),
                        )

                    return run

                units.append(xt_unit())
                for ft in range(12):
                    units.append(qk_unit(ft))
                for tt, (t0, tn) in enumerate(TT):
                    units.append(v_unit(tt, t0, tn))
                return units

            def emit_head_pair(b, hp, attn_out, filler=lambda: None):
                st = state[b]
                qkT, vext = st["qkT"], st["vext"]
                qf = hp
                kf = 6 + hp
                a_tiles = []
                for jt, (j0, jn) in enumerate(TT):
                    pds = [
                        pdp.tile([128, 2, 512], f32, tag="pd", name="pd_e"),
                        pdp.tile([128, 2, 512], f32, tag="pd", name="pd_o"),
                    ]
                    # even head on PE rows 0-63, odd head on rows 64-127:
                    # adjacent issue -> concurrent execution (row tiling)
                    for c in range(2):
                        for par in (0, 1):
                            r0 = par * 64
                            nc.tensor.matmul(
                                pds[par][0:jn, c, 0:CH],
                                qkT[r0 : r0 + 64, kf, j0 : j0 + jn],
                                qkT[r0 : r0 + 64, qf, c * CH : (c + 1) * CH],
                                start=True,
                                stop=True,
                            )
                    A2 = apl.tile([128, 2, NP], bf16, tag="A", name="A2")
                    # per-parity exp+mask so AV of the even head never waits
                    # on the odd head's exp
                    for par in (0, 1):
                        nc.scalar.activation(
                            out=A2[0:jn, par, :].rearrange("p (c i) -> p c i", c=2),
                            in_=pds[par][0:jn, :, 0:CH],
                            func=AF.Exp,
                        )
                        nc.vector.tensor_tensor(
                            out=A2[0:jn, par, j0 : j0 + jn],
                            in0=A2[0:jn, par, j0 : j0 + jn],
                            in1=mask[0:jn, 0:jn],
                            op=OP.mult,
                        )
                    a_tiles.append(A2)
                    filler()

                for par in (0, 1):
                    filler()
                    h = 2 * hp + par
                    r0 = par * 64
                    # odd head's accumulator comes from the pd pool (free once
                    # its exps are consumed) so AV_o never waits on the even
                    # head's normalize chain draining oe
                    if par == 0:
                        oe = pop.tile([128, 2, 512], f32, tag="oe")
                    else:
                        oe = pdp.tile([128, 2, 512], f32, tag="pd", name="oe_o")
                    for jt, (j0, jn) in enumerate(TT):
                        A2 = a_tiles[jt]
                        for c in range(2):
                            nc.tensor.matmul(
                                oe[0:128, c, 0:CH],
                                vext[0:jn, jt, h, :],
                                A2[0:jn, par, c * CH : (c + 1) * CH],
                                start=(jt == 0),
                                stop=(jt == len(TT) - 1),
                            )
                        if jt in (1, 3):
                            filler()
                    # custom-DVE ops misread PSUM partition offsets; stage the
                    # denominator rows into SBUF with tensor_copy first
                    den = smp.tile([128, NP], f32, tag="den", name="den")
                    nc.vector.tensor_copy(
                        out=den[0:64, :].rearrange("p (c i) -> p c i", c=2),
                        in_=oe[64:128, :, 0:CH],
                    )
                    recip = smp.tile([128, NP], f32, tag="recip")
                    nc.vector.reciprocal_approx_fast(
                        out=recip[0:64, :], in_=den[0:64, :]
                    )
                    nc.vector.tensor_tensor(
                        out=attn_out[r0 : r0 + 64, qf, :].rearrange(
                            "p (c i) -> p c i", c=2
                        ),
                        in0=oe[0:64, :, 0:CH],
                        in1=recip[0:64, :].rearrange("p (c i) -> p c i", c=2),
                        op=OP.mult,
                    )

            def outproj_units(b, attn_out):
                def unit(tt, t0, tn):
                    def run():
                        _outproj_tile(b, attn_out, tt, t0, tn)

                    return run

                return [unit(tt, t0, tn) for tt, (t0, tn) in enumerate(TT)]

            def _outproj_tile(b, attn_out, tt, t0, tn):
                po0 = pmp.tile([128, 512], f32, tag="pm", name="po0")
                po1 = pmp.tile([128, 512], f32, tag="pm", name="po1")
                for kc in range(6):
                    nc.tensor.matmul(
                        po0[0:tn, 0:512],
                        attn_out[:, kc, t0 : t0 + tn],
                        wout[:, kc, 0:512],
                        start=(kc == 0),
                        stop=(kc == 5),
                    )
                    nc.tensor.matmul(
                        po1[0:tn, 0:256],
                        attn_out[:, kc, t0 : t0 + tn],
                        wout[:, kc, 512:768],
                        start=(kc == 0),
                        stop=(kc == 5),
                    )
                ost = osp.tile([128, DIM], bf16)
                nc.vector.tensor_tensor(
                    out=ost[0:tn, 0:512],
                    in0=po0[0:tn, 0:512],
                    in1=b_bc[0:tn, 0:512],
                    op=OP.add,
                )
                nc.vector.tensor_tensor(
                    out=ost[0:tn, 512:768],
                    in0=po1[0:tn, 0:256],
                    in1=b_bc[0:tn, 512:768],
                    op=OP.add,
                )
                nc.sync.dma_start(
                    out=out_d[b, t0 : t0 + tn, :], in_=ost[0:tn, :]
                )

            # ---- software pipeline over batches ----
            # attention(b) interleaves prep(b+1); all deferred out-projections
            # (batches 0..BPC-2) fill the final batch's attention stream.
            units0 = prep_units(0)
            load_wqkv_block(0, 0)
            units0[0]()  # xT loads for batch 0 (sync queue)
            for kc in range(1, 6):
                load_wqkv_block(kc, 0)
            for blk in (1, 2):
                for kc in range(6):
                    load_wqkv_block(kc, blk)
            load_misc()
            for u in units0[1:]:
                u()
            load_wout()
            aouts = {}
            for b in range(BPC):
                units = prep_units(b + 1)
                if b == BPC - 1:
                    for pb in range(BPC - 1):
                        units = units + outproj_units(pb, aouts[pb])
                aouts[b] = aop.tile(
                    [128, 6, NP], bf16, tag="attn_out", name="attn_out"
                )
                # distribute filler units over the pairs' 7 fill points each
                fstate = {"ui": 0, "pt": 0}
                npoints = 6 * 11

                def filler():
                    fstate["pt"] += 1
                    want = min(
                        (len(units) * fstate["pt"]) // npoints, len(units)
                    )
                    while fstate["ui"] < want:
                        units[fstate["ui"]]()
                        fstate["ui"] += 1

                for hp in range(6):
                    emit_head_pair(b, hp, aouts[b], filler)
                while fstate["ui"] < len(units):
                    units[fstate["ui"]]()
                    fstate["ui"] += 1
            for u in outproj_units(BPC - 1, aouts[BPC - 1]):
                u()

    nc.compile()
    return nc


def _get_nc():
    if "nc" not in _cache:
        _cache["nc"] = _build_nc()
    return _cache["nc"]


def prepare_in_maps(inputs):
    import ml_dtypes

    bf = ml_dtypes.bfloat16
    x = np.asarray(inputs["x"], dtype=np.float32)
    W_qkv = np.asarray(inputs["W_qkv"], dtype=np.float32)
    scale = np.asarray(inputs["scale"], dtype=np.float32)
    W_out = np.ascontiguousarray(np.asarray(inputs["W_out"], dtype=np.float32))
    b_out = np.ascontiguousarray(np.asarray(inputs["b_out"], dtype=np.float32))

    # fold per-head LSA scale into the q columns of W_qkv
    Wq = W_qkv.copy()
    Wq[:, : H * DH] *= np.repeat(scale, DH)[None, :]
    Wq = np.ascontiguousarray(Wq.astype(bf))

    # host-transpose to [batch, feature, token], pad tokens to XW with zeros
    x_pad = np.zeros((B, DIM, XW), dtype=bf)
    x_pad[:, :, :N] = x.transpose(0, 2, 1).astype(bf)

    mask = np.ascontiguousarray((1.0 - np.eye(128, dtype=np.float32)).astype(bf))

    return [
        {
            "x": np.ascontiguousarray(x_pad[i * BPC : (i + 1) * BPC]),
            "wqkv": Wq,
            "wout": np.ascontiguousarray(W_out.astype(bf)),
            "bout": b_out,
            "mask": mask,
        }
        for i in range(NCORES)
    ]


def kernel(**inputs):
    from concourse import bass_utils

    nc = _get_nc()
    in_maps = prepare_in_maps(inputs)
    res = bass_utils.run_bass_kernel_spmd(nc, in_maps, core_ids=list(range(NCORES)))
    out = np.concatenate([res.results[i]["out"] for i in range(NCORES)], axis=0)
    return out.astype(np.float32)


# revision 39
# speedup vs baseline: 1.0046x; 1.0046x over previous
"""Trainium2 Bass kernel for ViT attention with LSA (per-head scale, masked diag).

Full inputs in, full outputs out. Sharding: data-parallel over batch across
8 NeuronCores (4 batches each). No collectives.

Per-core pipeline (per batch; x host-transposed + token-padded 577 -> 640):
  xT [768, 640]   plain contiguous DMA loads (transpose done on host)
  qT,kT = W_{q,k}^T @ x^T       (bf16 matmul -> bf16 tiles, feature-major,
                                 LSA scale folded into Wq on host)
  v     = x @ W_v               (token-major; vext[jt][h] = [v_h(64) | ones(64)])
  dotsT[j,i] = kT^T qT          (bf16 matmul K=64; heads 2m/2m+1 row-tiled at
                                 PE rows 0-63 / 64-127, issued adjacently so
                                 the two matmuls run concurrently ~2x)
  A = exp(dotsT)                (ACT, PSUM->SBUF bf16; no max-sub needed)
  A[diag block] *= (1 - I_128)  (DVE, only the diagonal 128-window per j-tile)
  oe[128, i] = vext^T A         (bf16 matmul; rows 64-127 = softmax denominator
                                 replicated via the 64 ones columns -- no
                                 cross-partition reduce or broadcast needed)
  attn_out = oe[0:64] * recip(oe[64:128])    (DVE f32 recip + mult -> bf16)
  out = attn_out^T @ W_out + b  (bf16 matmul, token-major; bf16 DMA out,
                                 upcast to f32 on host)

Batches are software-pipelined: batch b+1's projections (and, for the last
batch, all deferred out-projections) interleave into batch b's attention
stream at 9 fill points per head-pair to keep the PE dense (HAM warm).
The odd head's AV accumulator alternates into the pd pool so it never waits
on the even head's normalize chain.
"""

import numpy as np

B, N, DIM = 32, 577, 768
H, DH = 12, 64
NCORES = 8
BPC = B // NCORES          # batches per core
T = N                      # real tokens per batch
NP = 580                   # padded i-width (2 chunks of 290)
CH = 290                   # i-chunk size
XW = 640                   # DMA-transpose width (needs %128 free dim)
TT = [(i * 128, min(128, T - i * 128)) for i in range((T + 127) // 128)]  # j/t tiles

_cache = {}


def _build_nc():
    import concourse.bass as bass
    import concourse.tile as tile
    from concourse import bacc, mybir

    f32 = mybir.dt.float32
    bf16 = mybir.dt.bfloat16
    AF = mybir.ActivationFunctionType
    OP = mybir.AluOpType

    nc = bacc.Bacc("TRN2", target_bir_lowering=False, debug=False)

    # x arrives host-transposed: [batch, feature, token(padded to XW)]
    x_d = nc.dram_tensor("x", [BPC, DIM, XW], bf16, kind="ExternalInput").ap()
    wqkv_d = nc.dram_tensor("wqkv", [DIM, 3 * DIM], bf16, kind="ExternalInput").ap()
    wout_d = nc.dram_tensor("wout", [DIM, DIM], bf16, kind="ExternalInput").ap()
    bout_d = nc.dram_tensor("bout", [DIM], f32, kind="ExternalInput").ap()
    mask_d = nc.dram_tensor("mask", [128, 128], bf16, kind="ExternalInput").ap()
    out_d = nc.dram_tensor("out", [BPC, N, DIM], bf16, kind="ExternalOutput").ap()

    with tile.TileContext(nc) as tc:
        with (
            tc.tile_pool(name="weights", bufs=1) as wp,
            tc.tile_pool(name="xt", bufs=3) as xtp,
            tc.tile_pool(name="qkt", bufs=2) as qkp,
            tc.tile_pool(name="vext", bufs=2) as vp,
            tc.tile_pool(name="aout", bufs=4) as aop,
            tc.tile_pool(name="apool", bufs=6) as apl,
            tc.tile_pool(name="small", bufs=2) as smp,
            tc.tile_pool(name="ostage", bufs=2) as osp,
            tc.tile_pool(name="pmisc", bufs=2, space="PSUM") as pmp,
            tc.tile_pool(name="pdots", bufs=2, space="PSUM") as pdp,
            tc.tile_pool(name="poext", bufs=1, space="PSUM") as pop,
        ):
            # ---- static tiles ----
            wqkv = wp.tile([128, 6, 3 * DIM], bf16)

            # weight loads ride the otherwise-idle gpsimd SWDGE queue so they
            # never serialize behind xT / output traffic or a busy engine FIFO
            def load_wqkv_block(kc, blk, eng=None):
                (eng or nc.gpsimd).dma_start(
                    out=wqkv[:, kc, blk * DIM : (blk + 1) * DIM],
                    in_=wqkv_d[kc * 128 : (kc + 1) * 128, blk * DIM : (blk + 1) * DIM],
                )

            wout = wp.tile([128, 6, DIM], bf16)

            def load_wout():
                for kc in range(6):
                    nc.gpsimd.dma_start(
                        out=wout[:, kc, :], in_=wout_d[kc * 128 : (kc + 1) * 128, :]
                    )

            mask = wp.tile([128, 128], bf16)
            b_bc = wp.tile([128, DIM], f32)

            def load_misc():
                nc.gpsimd.dma_start(out=mask, in_=mask_d)
                bout_bcast = bass.AP(
                    tensor=bout_d.tensor,
                    offset=bout_d.offset,
                    ap=[[0, 128], [1, DIM]],
                )
                nc.gpsimd.dma_start(out=b_bc, in_=bout_bcast)

            state = {}  # b -> dict(xT=, qkT=, vext=)

            def prep_units(b):
                """Thunks for batch b's load/projection work."""
                if b >= BPC:
                    return []
                st = {}
                state[b] = st
                units = []

                def xt_unit():
                    def run():
                        st["xT"] = xtp.tile([128, 6, XW], bf16, tag="xT", name="xT")
                        for dc in range(6):
                            nc.sync.dma_start(
                                out=st["xT"][:, dc, :],
                                in_=x_d[b, dc * 128 : (dc + 1) * 128, :],
                            )

                    return run

                def qk_unit(ft):
                    # c-inner: consecutive matmuls alternate PSUM banks so
                    # drain of one overlaps fill of the other
                    def run():
                        if "qkT" not in st:
                            st["qkT"] = qkp.tile(
                                [128, 12, NP], bf16, tag="qkT", name="qkT"
                            )
                        xT, qkT = st["xT"], st["qkT"]
                        pq = [
                            pmp.tile([128, 512], f32, tag="pm", name="pq0"),
                            pmp.tile([128, 512], f32, tag="pm", name="pq1"),
                        ]
                        for kc in range(6):
                            for c in range(2):
                                nc.tensor.matmul(
                                    pq[c][:, 0:CH],
                                    wqkv[:, kc, ft * 128 : (ft + 1) * 128],
                                    xT[:, kc, c * CH : (c + 1) * CH],
                                    start=(kc == 0),
                                    stop=(kc == 5),
                                )
                        nc.vector.tensor_copy(
                            out=qkT[:, ft, 0:CH], in_=pq[0][:, 0:CH]
                        )
                        nc.scalar.copy(
                            out=qkT[:, ft, CH : 2 * CH], in_=pq[1][:, 0:CH]
                        )

                    return run

                def v_unit(tt, t0, tn):
                    def run():
                        if "vext" not in st:
                            st["vext"] = vp.tile(
                                [128, len(TT), 12, 128], bf16, tag="vext",
                                name="vext",
                            )
                            # ones block: cols 64-127 of every (tt, h) slice;
                            # AV's ones columns replicate the softmax
                            # denominator onto PSUM partitions 64-127.
                            nc.vector.memset(st["vext"][:, :, :, 64:128], 1.0)
                        xT, vext = st["xT"], st["vext"]
                        pv0 = pmp.tile([128, 512], f32, tag="pm", name="pv0")
                        pv1 = pmp.tile([128, 512], f32, tag="pm", name="pv1")
                        for kc in range(6):
                            nc.tensor.matmul(
                                pv0[0:tn, 0:512],
                                xT[:, kc, t0 : t0 + tn],
                                wqkv[:, kc, 1536:2048],
                                start=(kc == 0),
                                stop=(kc == 5),
                            )
                            nc.tensor.matmul(
                                pv1[0:tn, 0:256],
                                xT[:, kc, t0 : t0 + tn],
                                wqkv[:, kc, 2048:2304],
                                start=(kc == 0),
                                stop=(kc == 5),
                            )
                        nc.vector.tensor_copy(
                            out=vext[0:tn, tt, 0:8, 0:DH],
                            in_=pv0[0:tn, 0:512].rearrange("p (h d) -> p h d", h=8),
                        )
                        nc.vector.tensor_copy(
                            out=vext[0:tn, tt, 8:12, 0:DH],
                            in_=pv1[0:tn, 0:256].rearrange("p (h d) -> p h d", h=4),
                        )

                    return run

                units.append(xt_unit())
                for ft in range(12):
                    units.append(qk_unit(ft))
                for tt, (t0, tn) in enumerate(TT):
                    units.append(v_unit(tt, t0, tn))
                return units

            def emit_head_pair(b, hp, attn_out, filler=lambda: None):
                st = state[b]
                qkT, vext = st["qkT"], st["vext"]
                qf = hp
                kf = 6 + hp
                a_tiles = []
                for jt, (j0, jn) in enumerate(TT):
                    pds = [
                        pdp.tile([128, 2, 512], f32, tag="pd", name="pd_e"),
                        pdp.tile([128, 2, 512], f32, tag="pd", name="pd_o"),
                    ]
                    # even head on PE rows 0-63, odd head on rows 64-127:
                    # adjacent issue -> concurrent execution (row tiling)
                    for c in range(2):
                        for par in (0, 1):
                            r0 = par * 64
                            nc.tensor.matmul(
                                pds[par][0:jn, c, 0:CH],
                                qkT[r0 : r0 + 64, kf, j0 : j0 + jn],
                                qkT[r0 : r0 + 64, qf, c * CH : (c + 1) * CH],
                                start=True,
                                stop=True,
                            )
                    A2 = apl.tile([128, 2, NP], bf16, tag="A", name="A2")
                    # per-parity exp+mask so AV of the even head never waits
                    # on the odd head's exp
                    for par in (0, 1):
                        nc.scalar.activation(
                            out=A2[0:jn, par, :].rearrange("p (c i) -> p c i", c=2),
                            in_=pds[par][0:jn, :, 0:CH],
                            func=AF.Exp,
                        )
                        nc.vector.tensor_tensor(
                            out=A2[0:jn, par, j0 : j0 + jn],
                            in0=A2[0:jn, par, j0 : j0 + jn],
                            in1=mask[0:jn, 0:jn],
                            op=OP.mult,
                        )
                    a_tiles.append(A2)
                    filler()

                for par in (0, 1):
                    filler()
                    h = 2 * hp + par
                    r0 = par * 64
                    # odd head's accumulator comes from the pd pool (free once
                    # its exps are consumed) so AV_o never waits on the even
                    # head's normalize chain draining oe
                    if par == 0:
                        oe = pop.tile([128, 2, 512], f32, tag="oe")
                    else:
                        oe = pdp.tile([128, 2, 512], f32, tag="pd", name="oe_o")
                    for jt, (j0, jn) in enumerate(TT):
                        A2 = a_tiles[jt]
                        for c in range(2):
                            nc.tensor.matmul(
                                oe[0:128, c, 0:CH],
                                vext[0:jn, jt, h, :],
                                A2[0:jn, par, c * CH : (c + 1) * CH],
                                start=(jt == 0),
                                stop=(jt == len(TT) - 1),
                            )
                        if jt in (1, 3):
                            filler()
                    # custom-DVE ops misread PSUM partition offsets; stage the
                    # denominator rows into SBUF with tensor_copy first
                    den = smp.tile([128, NP], f32, tag="den", name="den")
                    nc.vector.tensor_copy(
                        out=den[0:64, :].rearrange("p (c i) -> p c i", c=2),
                        in_=oe[64:128, :, 0:CH],
                    )
                    recip = smp.tile([128, NP], f32, tag="recip")
                    nc.vector.reciprocal_approx_fast(
                        out=recip[0:64, :], in_=den[0:64, :]
                    )
                    nc.vector.tensor_tensor(
                        out=attn_out[r0 : r0 + 64, qf, :].rearrange(
                            "p (c i) -> p c i", c=2
                        ),
                        in0=oe[0:64, :, 0:CH],
                        in1=recip[0:64, :].rearrange("p (c i) -> p c i", c=2),
                        op=OP.mult,
                    )

            def outproj_units(b, attn_out, use_pd=False):
                def unit(tt, t0, tn):
                    def run():
                        _outproj_tile(b, attn_out, tt, t0, tn, use_pd)

                    return run

                return [unit(tt, t0, tn) for tt, (t0, tn) in enumerate(TT)]

            def _outproj_tile(b, attn_out, tt, t0, tn, use_pd=False):
                if use_pd:
                    # post-attention: the dots pool is idle -- use it so the
                    # tail out-projections don't wait on pm-pool recycling
                    po = pdp.tile([128, 2, 512], f32, tag="pd", name="pofin")
                    po0, po1 = po[:, 0, :], po[:, 1, :]
                else:
                    po0 = pmp.tile([128, 512], f32, tag="pm", name="po0")
                    po1 = pmp.tile([128, 512], f32, tag="pm", name="po1")
                for kc in range(6):
                    nc.tensor.matmul(
                        po0[0:tn, 0:512],
                        attn_out[:, kc, t0 : t0 + tn],
                        wout[:, kc, 0:512],
                        start=(kc == 0),
                        stop=(kc == 5),
                    )
                    nc.tensor.matmul(
                        po1[0:tn, 0:256],
                        attn_out[:, kc, t0 : t0 + tn],
                        wout[:, kc, 512:768],
                        start=(kc == 0),
                        stop=(kc == 5),
                    )
                ost = osp.tile([128, DIM], bf16)
                nc.vector.tensor_tensor(
                    out=ost[0:tn, 0:512],
                    in0=po0[0:tn, 0:512],
                    in1=b_bc[0:tn, 0:512],
                    op=OP.add,
                )
                nc.vector.tensor_tensor(
                    out=ost[0:tn, 512:768],
                    in0=po1[0:tn, 0:256],
                    in1=b_bc[0:tn, 512:768],
                    op=OP.add,
                )
                nc.sync.dma_start(
                    out=out_d[b, t0 : t0 + tn, :], in_=ost[0:tn, :]
                )

            # ---- software pipeline over batches ----
            # attention(b) interleaves prep(b+1); all deferred out-projections
            # (batches 0..BPC-2) fill the final batch's attention stream.
            units0 = prep_units(0)
            load_wqkv_block(0, 0)
            units0[0]()  # xT loads for batch 0 (sync queue)
            for kc in range(1, 6):
                load_wqkv_block(kc, 0)
            for blk in (1, 2):
                for kc in range(6):
                    load_wqkv_block(kc, blk)
            load_misc()
            for u in units0[1:]:
                u()
            load_wout()
            aouts = {}
            for b in range(BPC):
                units = prep_units(b + 1)
                if b == BPC - 1:
                    for pb in range(BPC - 1):
                        units = units + outproj_units(pb, aouts[pb])
                aouts[b] = aop.tile(
                    [128, 6, NP], bf16, tag="attn_out", name="attn_out"
                )
                # distribute filler units over the pairs' 7 fill points each
                fstate = {"ui": 0, "pt": 0}
                npoints = 6 * 11

                def filler():
                    fstate["pt"] += 1
                    want = min(
                        (len(units) * fstate["pt"]) // npoints, len(units)
                    )
                    while fstate["ui"] < want:
                        units[fstate["ui"]]()
                        fstate["ui"] += 1

                for hp in range(6):
                    emit_head_pair(b, hp, aouts[b], filler)
                while fstate["ui"] < len(units):
                    units[fstate["ui"]]()
                    fstate["ui"] += 1
            for u in outproj_units(BPC - 1, aouts[BPC - 1], use_pd=True):
                u()

    nc.compile()
    return nc


def _get_nc():
    if "nc" not in _cache:
        _cache["nc"] = _build_nc()
    return _cache["nc"]


def prepare_in_maps(inputs):
    import ml_dtypes

    bf = ml_dtypes.bfloat16
    x = np.asarray(inputs["x"], dtype=np.float32)
    W_qkv = np.asarray(inputs["W_qkv"], dtype=np.float32)
    scale = np.asarray(inputs["scale"], dtype=np.float32)
    W_out = np.ascontiguousarray(np.asarray(inputs["W_out"], dtype=np.float32))
    b_out = np.ascontiguousarray(np.asarray(inputs["b_out"], dtype=np.float32))

    # fold per-head LSA scale into the q columns of W_qkv
    Wq = W_qkv.copy()
    Wq[:, : H * DH] *= np.repeat(scale, DH)[None, :]
    Wq = np.ascontiguousarray(Wq.astype(bf))

    # host-transpose to [batch, feature, token], pad tokens to XW with zeros
    x_pad = np.zeros((B, DIM, XW), dtype=bf)
    x_pad[:, :, :N] = x.transpose(0, 2, 1).astype(bf)

    mask = np.ascontiguousarray((1.0 - np.eye(128, dtype=np.float32)).astype(bf))

    return [
        {
            "x": np.ascontiguousarray(x_pad[i * BPC : (i + 1) * BPC]),
            "wqkv": Wq,
            "wout": np.ascontiguousarray(W_out.astype(bf)),
            "bout": b_out,
            "mask": mask,
        }
        for i in range(NCORES)
    ]


def kernel(**inputs):
    from concourse import bass_utils

    nc = _get_nc()
    in_maps = prepare_in_maps(inputs)
    res = bass_utils.run_bass_kernel_spmd(nc, in_maps, core_ids=list(range(NCORES)))
    out = np.concatenate([res.results[i]["out"] for i in range(NCORES)], axis=0)
    return out.astype(np.float32)
